# revision 1
# baseline (speedup 1.0000x reference)
"""Self-contained Trainium2 Bass kernel for nn_AttentionBlock (GroupNorm +
self/cross attention + projection + residual), data-parallel over batch on
8 NeuronCores.

kernel(**inputs) takes the FULL unsharded inputs of reference.setup_inputs()
and returns the FULL [16, 512, 32, 32] fp32 output.

v2 design (item-pipelined, engine-balanced):
- Data-parallel: 16 batch items / 8 cores = 2 per core; weights replicated.
- The per-core program is ONE software-pipelined stream over items
  (rep-unrolled): while item k's attention runs (ACT-bound: 72 exp
  instructions), the PE idle gaps are filled with item k+1's GroupNorm /
  qkv / v matmuls and item k-1's projection, pulled from a unit queue.
- Attention math per (head-pair, key-chunk) event:
  scores row-packed (two K=64 heads at PE rows 0/64), exp on ACT,
  value matmul COL-TILED (two M=64 heads at PE cols 0/64 writing one
  [128, T] fp32 PSUM accumulator - full array utilization), and compact
  denominator rows via M=1 col-tiled all-ones matmuls accumulated at
  PSUM partitions {0,32}/{64,96} of a per-head-pair tile.
- Normalization off the critical path: accumulators are evacuated to SBUF
  (bf16) immediately (freeing PSUM for the next head-pair), denominators
  reciprocated once per head-pair (one DVE op covers 4 rows), broadcast
  via K=1 PE matmuls, multiplied in one DVE op per head-pair-half.
- GroupNorm rstd uses exp(-0.5*ln(var+eps)) so the ACT engine only ever
  needs the {exp, ln} table set: zero activation-table reloads.
- PSUM: scores ring 2x2 banks + value accumulator 2 + denominators 2 = 8.
"""
import math
from collections import deque

import numpy as np
import concourse.bass as bass
import concourse.mybir as mybir
import concourse.tile as tile
from concourse.bass_utils import run_bass_kernel_spmd

B, C, HS, WS = 16, 512, 32, 32
T = HS * WS          # 1024 spatial positions
HEADS, HC = 8, 64
L = 77               # cond sequence length
CD = 768             # cond dim
S = T + L            # 1101 key positions
GROUPS = 32
GSIZE = C // GROUPS  # 16 channels per group
EPS = 1e-5
N_CORES = 8
BPC = B // N_CORES   # batch items per core
CT = C // 128        # 4 channel tiles
CCT = CD // 128      # 6 cond channel tiles
NSC = 9              # key chunks: 8 x 128 + 77
TH = 2               # t halves of 512
LAG = 3              # events between scores/exp and value/den consumption

F16 = mybir.dt.float16
BF16 = mybir.dt.bfloat16
F32 = mybir.dt.float32
AF = mybir.ActivationFunctionType
AL = mybir.AluOpType

_CACHE = {}


def split_multi_waits(nc):
    """walrus in this container accepts at most one sync wait per
    instruction; hoist extra waits onto preceding NOPs on the same engine."""
    n_split = 0
    for f in nc.m.functions:
        for blk in f.blocks:
            new_insts = []
            for inst in blk.instructions:
                si = inst.sync_info
                if si is not None and si.on_wait is not None and len(si.on_wait) > 1:
                    waits = list(si.on_wait)
                    for w in waits[:-1]:
                        nop = mybir.InstNoOp(
                            name=f"{inst.name}-wsplit{n_split}",
                            ins=[], outs=[],
                            sync_info=mybir.SyncInfo(on_wait=[w], on_update=[]),
                        )
                        nop.engine = inst.engine
                        new_insts.append(nop)
                        n_split += 1
                    si.on_wait = [waits[-1]]
                    inst.sync_info = si
                new_insts.append(inst)
            blk.instructions = new_insts
    return n_split


def build_program(apply_vbias=False, apply_pbias=False, repeat=1):
    nc = bass.Bass("TRN2", target_bir_lowering=False, debug=False, num_devices=1)

    xd = nc.dram_tensor("x_sh", [BPC, C, T], F32, kind="ExternalInput")
    cd = nc.dram_tensor("c_sh", [BPC, CD, L], F16, kind="ExternalInput")
    wqd = nc.dram_tensor("wqT", [C, C], F16, kind="ExternalInput")
    wkd = nc.dram_tensor("wkT", [C, C], F16, kind="ExternalInput")
    wvd = nc.dram_tensor("wvT", [C, C], F16, kind="ExternalInput")
    wkcd = nc.dram_tensor("wkcT", [CD, C], F16, kind="ExternalInput")
    wvcd = nc.dram_tensor("wvcT", [CD, C], F16, kind="ExternalInput")
    wpd = nc.dram_tensor("wpT", [C, C], F16, kind="ExternalInput")
    Gd = nc.dram_tensor("G", [128, CT, GROUPS], F32, kind="ExternalInput")
    GTd = nc.dram_tensor("GT", [GROUPS, CT, 128], F32, kind="ExternalInput")
    qbd = nc.dram_tensor("qb", [128, CT], F32, kind="ExternalInput")
    kbd = nc.dram_tensor("kb", [128, CT], F32, kind="ExternalInput")
    kcbd = nc.dram_tensor("kcb", [128, CT], F32, kind="ExternalInput")
    pbd = nc.dram_tensor("pb", [128, CT], F32, kind="ExternalInput")
    vbd = nc.dram_tensor("vbrow", [1, HEADS * HC], F16, kind="ExternalInput")
    outd = nc.dram_tensor("out", [BPC, C, T], F32, kind="ExternalOutput")

    NITEMS = repeat * BPC

    with tile.TileContext(nc) as tc:
        with tc.tile_pool(name="wp", bufs=1) as wp, \
             tc.tile_pool(name="xp", bufs=1) as xp, \
             tc.tile_pool(name="gp", bufs=1) as gp, \
             tc.tile_pool(name="qp", bufs=1) as qp, \
             tc.tile_pool(name="ap", bufs=1) as app, \
             tc.tile_pool(name="psp", bufs=1, space="PSUM") as psp:

            st = [dict(x={}, E={}, xn={}, q={}, k={}, vt={}, apair={},
                       araw={}, acc={}, den={}, rc={})
                  for _ in range(NITEMS)]

            def emit_input_dmas(k):
                b = k % BPC
                for m in range(CT):
                    t_ = xp.tile([128, T], F32, name=f"x_{k}_{m}",
                                 tag="x", bufs=10)
                    nc.sync.dma_start(t_[:], xd.ap()[b, 128 * m:128 * (m + 1), :])
                    st[k]["x"][m] = t_
                t_ = xp.tile([128, CCT, L], F16, name=f"c_{k}", tag="c", bufs=3)
                nc.sync.dma_start(t_[:],
                                  cd.ap()[b].rearrange("(a p) l -> p a l", p=128))
                st[k]["c"] = t_

            # input DMAs for the first two items go out before the big
            # weight DMAs: the GroupNorm stats chain gates startup.
            emit_input_dmas(0)
            if NITEMS > 1:
                emit_input_dmas(1)

            # ---- weights & constants (G/GT/biases first: needed earliest)
            G_sb = wp.tile([128, CT, GROUPS], F32, name="G_sb")
            GT_sb = wp.tile([GROUPS, CT, 128], F32, name="GT_sb")
            nc.sync.dma_start(G_sb[:], Gd.ap())
            nc.sync.dma_start(GT_sb[:], GTd.ap())
            qb_sb = wp.tile([128, CT], F32, name="qb_sb")
            kb_sb = wp.tile([128, CT], F32, name="kb_sb")
            kcb_sb = wp.tile([128, CT], F32, name="kcb_sb")
            pb_sb = wp.tile([128, CT], F32, name="pb_sb")
            nc.sync.dma_start(qb_sb[:], qbd.ap())
            nc.sync.dma_start(kb_sb[:], kbd.ap())
            nc.sync.dma_start(kcb_sb[:], kcbd.ap())
            nc.sync.dma_start(pb_sb[:], pbd.ap())
            wq_sb = wp.tile([128, CT, C], F16, name="wq_sb")
            wk_sb = wp.tile([128, CT, C], F16, name="wk_sb")
            wkc_sb = wp.tile([128, CCT, C], F16, name="wkc_sb")
            wv_sb = wp.tile([128, CT, C], F16, name="wv_sb")
            wvc_sb = wp.tile([128, CCT, C], F16, name="wvc_sb")
            wp_sb = wp.tile([128, CT, C], F16, name="wp_sb")
            nc.sync.dma_start(wq_sb[:], wqd.ap().rearrange("(a p) o -> p a o", p=128))
            nc.sync.dma_start(wk_sb[:], wkd.ap().rearrange("(a p) o -> p a o", p=128))
            nc.sync.dma_start(wkc_sb[:], wkcd.ap().rearrange("(a p) o -> p a o", p=128))
            nc.sync.dma_start(wv_sb[:], wvd.ap().rearrange("(a p) o -> p a o", p=128))
            nc.sync.dma_start(wvc_sb[:], wvcd.ap().rearrange("(a p) o -> p a o", p=128))
            nc.sync.dma_start(wp_sb[:], wpd.ap().rearrange("(a p) o -> p a o", p=128))
            ones1 = wp.tile([128, 1], F16, name="ones1")
            nc.vector.memset(ones1[:], 1.0)
            ones64 = wp.tile([128, HC], F16, name="ones64")
            nc.vector.memset(ones64[:], 1.0)
            eps_sb = wp.tile([GROUPS, 1], F32, name="eps_sb")
            nc.vector.memset(eps_sb[:], EPS)
            zero_g = wp.tile([GROUPS, 1], F32, name="zero_g")
            nc.vector.memset(zero_g[:], 0.0)
            if apply_vbias:
                vb_bc = wp.tile([128, HEADS * HC], F16, name="vb_bc")
                nc.gpsimd.dma_start(
                    out=vb_bc[:],
                    in_=bass.AP(tensor=vbd.ap().tensor, offset=0,
                                ap=[[0, 128], [1, HEADS * 65]]),
                )

            pending = deque()

            def drain(n):
                for _ in range(min(n, len(pending))):
                    pending.popleft()()

            # ---------------- per-item unit builders ----------------------
            def push_prep(k):
                """GroupNorm + xn + q/k/kc + v units for item k."""
                def u_gn_stats():
                    s12 = {}
                    for m in range(CT):
                        stt = gp.tile([128, 2, 6], F32, name=f"bnst_{k}_{m}",
                                      tag="bnst", bufs=8)
                        for sg in range(2):
                            nc.vector.bn_stats(
                                out=stt[:, sg, :],
                                in_=st[k]["x"][m][:, 512 * sg:512 * (sg + 1)])
                        mv = gp.tile([128, 2], F32, name=f"mv_{k}_{m}",
                                     tag="mv", bufs=8)
                        nc.vector.bn_aggr(out=mv[:], in_=stt[:])
                        s12t = gp.tile([128, 2], F32, name=f"s12_{k}_{m}",
                                       tag="s12", bufs=8)
                        nc.vector.tensor_copy(s12t[:, 0:1], mv[:, 0:1])
                        nc.vector.tensor_scalar(
                            out=s12t[:, 1:2], in0=mv[:, 0:1], scalar1=mv[:, 0:1],
                            scalar2=None, op0=AL.mult)
                        nc.vector.tensor_tensor(
                            out=s12t[:, 1:2], in0=s12t[:, 1:2], in1=mv[:, 1:2],
                            op=AL.add)
                        s12[m] = s12t
                    st_ps = psp.tile([GROUPS, 2], F32, name=f"stps_{k}", tag="sc",
                                     bufs=2, padded_shape=[128, T])
                    for m in range(CT):
                        nc.tensor.matmul(st_ps[:], G_sb[:, m, :], s12[m][:],
                                         start=(m == 0), stop=(m == CT - 1))
                    grp = gp.tile([GROUPS, 6], F32, name=f"grp_{k}",
                                  tag="grp", bufs=4)
                    nc.vector.tensor_copy(grp[:, 4:6], st_ps[:])
                    # cols: 0=mean 1=rstd 2=mean 3=var 4=gsum_mean 5=gsum_e2
                    nc.vector.tensor_scalar_mul(grp[:, 2:3], in0=grp[:, 4:5],
                                                scalar1=1.0 / GSIZE)
                    nc.vector.tensor_scalar_mul(grp[:, 3:4], in0=grp[:, 5:6],
                                                scalar1=1.0 / GSIZE)
                    nc.vector.tensor_scalar(out=grp[:, 1:2], in0=grp[:, 2:3],
                                            scalar1=grp[:, 2:3], scalar2=None,
                                            op0=AL.mult)
                    nc.vector.tensor_tensor(out=grp[:, 3:4], in0=grp[:, 3:4],
                                            in1=grp[:, 1:2], op=AL.subtract)
                    # rstd = exp(-0.5*ln(var+eps)): stays on the exp/ln ACT
                    # table set (a Sqrt would force a table reload mid-stream)
                    nc.scalar.activation(grp[:, 3:4], grp[:, 3:4], AF.Ln,
                                         bias=eps_sb[:])
                    nc.scalar.activation(grp[:, 1:2], grp[:, 3:4], AF.Exp,
                                         bias=zero_g[:], scale=-0.5)
                    nc.vector.tensor_copy(grp[:, 0:1], grp[:, 2:3])
                    for m in range(CT):
                        e_ps = psp.tile([128, 2], F32, name=f"eps_{k}_{m}",
                                        tag="sc", bufs=2, padded_shape=[128, T])
                        nc.tensor.matmul(e_ps[:], GT_sb[:, m, :], grp[:, 0:2],
                                         start=True, stop=True)
                        e_t = gp.tile([128, 2], F32, name=f"E_{k}_{m}",
                                      tag="E", bufs=8)
                        nc.vector.tensor_copy(e_t[:], e_ps[:])
                        st[k]["E"][m] = e_t
                pending.append(u_gn_stats)

                def mk_xn(m):
                    def u():
                        xnt = qp.tile([128, T], F16, name=f"xn_{k}_{m}",
                                      tag="xn", bufs=8)
                        nc.vector.tensor_scalar(
                            out=xnt[:], in0=st[k]["x"][m][:],
                            scalar1=st[k]["E"][m][:, 0:1],
                            scalar2=st[k]["E"][m][:, 1:2],
                            op0=AL.subtract, op1=AL.mult)
                        st[k]["xn"][m] = xnt
                    return u
                for m in range(CT):
                    pending.append(mk_xn(m))

                def mk_q(m):
                    def u():
                        q_ps = psp.tile([128, T], F32, name=f"qps_{k}_{m}",
                                        tag="sc", bufs=2)
                        for ci in range(CT):
                            for th in range(TH):
                                nc.tensor.matmul(
                                    q_ps[:, 512 * th:512 * (th + 1)],
                                    wq_sb[:, ci, 128 * m:128 * (m + 1)],
                                    st[k]["xn"][ci][:, 512 * th:512 * (th + 1)],
                                    start=(ci == 0), stop=(ci == CT - 1))
                        qt = qp.tile([128, T], F16, name=f"q_{k}_{m}",
                                     tag="q", bufs=8)
                        nc.vector.tensor_scalar(out=qt[:], in0=q_ps[:],
                                                scalar1=qb_sb[:, m:m + 1],
                                                scalar2=None, op0=AL.add)
                        st[k]["q"][m] = qt
                    return u

                def mk_k(m):
                    def u():
                        k_ps = psp.tile([128, T], F32, name=f"kps_{k}_{m}",
                                        tag="sc", bufs=2)
                        for ci in range(CT):
                            for th in range(TH):
                                nc.tensor.matmul(
                                    k_ps[:, 512 * th:512 * (th + 1)],
                                    wk_sb[:, ci, 128 * m:128 * (m + 1)],
                                    st[k]["xn"][ci][:, 512 * th:512 * (th + 1)],
                                    start=(ci == 0), stop=(ci == CT - 1))
                        kt = qp.tile([128, S], F16, name=f"k_{k}_{m}",
                                     tag="k", bufs=8)
                        nc.vector.tensor_scalar(out=kt[:, 0:T], in0=k_ps[:],
                                                scalar1=kb_sb[:, m:m + 1],
                                                scalar2=None, op0=AL.add)
                        st[k]["k"][m] = kt
                    return u

                def mk_kc(m):
                    def u():
                        kc_ps = psp.tile([128, L], F32, name=f"kcps_{k}_{m}",
                                         tag="sc", bufs=2, padded_shape=[128, T])
                        for ci in range(CCT):
                            nc.tensor.matmul(kc_ps[:],
                                             wkc_sb[:, ci, 128 * m:128 * (m + 1)],
                                             st[k]["c"][:, ci, :],
                                             start=(ci == 0), stop=(ci == CCT - 1))
                        nc.vector.tensor_scalar(out=st[k]["k"][m][:, T:S],
                                                in0=kc_ps[:],
                                                scalar1=kcb_sb[:, m:m + 1],
                                                scalar2=None, op0=AL.add)
                    return u

                for m in range(CT):
                    pending.append(mk_q(m))
                    pending.append(mk_k(m))
                    pending.append(mk_kc(m))

                def mk_v(s):
                    def u():
                        sdim = 128 if s < 8 else L
                        vt = qp.tile([sdim, HEADS * HC], F16, name=f"vt_{k}_{s}",
                                     tag="vt", bufs=18,
                                     padded_shape=[128, HEADS * HC])
                        pv = psp.tile([sdim, 512], F32, name=f"pv_{k}_{s}",
                                      tag="sc", bufs=2, padded_shape=[128, T])
                        if s < 8:
                            for ci in range(CT):
                                nc.tensor.matmul(
                                    pv[:],
                                    st[k]["xn"][ci][:, 128 * s:128 * (s + 1)],
                                    wv_sb[:, ci, :],
                                    start=(ci == 0), stop=(ci == CT - 1))
                        else:
                            for ci in range(CCT):
                                nc.tensor.matmul(
                                    pv[:], st[k]["c"][:, ci, :], wvc_sb[:, ci, :],
                                    start=(ci == 0), stop=(ci == CCT - 1))
                        nc.vector.tensor_copy(vt[:], pv[:])
                        if apply_vbias:
                            nc.vector.tensor_tensor(
                                out=vt[:], in0=vt[:], in1=vb_bc[:sdim, :],
                                op=AL.add)
                        st[k]["vt"][s] = vt
                    return u
                for s in range(NSC):
                    pending.append(mk_v(s))

            def push_proj(k):
                b = k % BPC

                def mk_proj(m):
                    def u():
                        h_ps = psp.tile([128, T], F32, name=f"hps_{k}_{m}",
                                        tag="sc", bufs=2)
                        for ci in range(CT):
                            for th in range(TH):
                                tsl = slice(512 * th, 512 * (th + 1))
                                nc.tensor.matmul(
                                    h_ps[:, tsl],
                                    wp_sb[:, ci, 128 * m:128 * (m + 1)],
                                    st[k]["apair"][ci][:, tsl],
                                    start=(ci == 0), stop=(ci == CT - 1))
                        xt = st[k]["x"][m]
                        nc.vector.tensor_tensor(out=xt[:], in0=h_ps[:],
                                                in1=xt[:], op=AL.add)
                        if apply_pbias:
                            nc.vector.tensor_scalar(out=xt[:], in0=xt[:],
                                                    scalar1=pb_sb[:, m:m + 1],
                                                    scalar2=None, op0=AL.add)
                        nc.sync.dma_start(
                            outd.ap()[b, 128 * m:128 * (m + 1), :], xt[:])
                    return u
                for m in range(CT):
                    pending.append(mk_proj(m))

            # ---------------- attention event stream -----------------------
            def run_attention(k):
                events = [(hp, s) for hp in range(CT) for s in range(NSC)]
                pt = {}

                def emit_scores(hp, s):
                    sdim = 128 if s < 8 else L
                    ssl = slice(128 * s, 128 * s + sdim) if s < 8 else slice(T, S)
                    kt = st[k]["k"][hp]
                    qt = st[k]["q"][hp]
                    sc_e = psp.tile([sdim, T], F32, name=f"sce_{k}_{hp}_{s}",
                                    tag="sc", bufs=2, padded_shape=[128, T])
                    sc_o = psp.tile([sdim, T], F32, name=f"sco_{k}_{hp}_{s}",
                                    tag="sc", bufs=2, padded_shape=[128, T])
                    for th in range(TH):
                        tsl = slice(512 * th, 512 * (th + 1))
                        nc.tensor.matmul(sc_e[:, tsl], kt[0:64, ssl],
                                         qt[0:64, tsl], start=True, stop=True,
                                         tile_position=(0, 0))
                        nc.tensor.matmul(sc_o[:, tsl], kt[64:128, ssl],
                                         qt[64:128, tsl], start=True, stop=True,
                                         tile_position=(64, 0))
                    pe = qp.tile([sdim, T], F16, name=f"pte_{k}_{hp}_{s}",
                                 tag="pt", bufs=8, padded_shape=[128, T])
                    po = qp.tile([sdim, T], F16, name=f"pto_{k}_{hp}_{s}",
                                 tag="pt", bufs=8, padded_shape=[128, T])
                    nc.scalar.activation(pe[:], sc_e[:], AF.Exp)
                    nc.scalar.activation(po[:], sc_o[:], AF.Exp)
                    pt[(hp, s, 0)] = pe
                    pt[(hp, s, 1)] = po

                def emit_value_den(hp, s):
                    sdim = 128 if s < 8 else L
                    pe = pt.pop((hp, s, 0))
                    po = pt.pop((hp, s, 1))
                    if s == 0:
                        st[k]["acc"][hp] = psp.tile(
                            [128, T], F32, name=f"acc_{k}_{hp}", tag="acc",
                            bufs=1)
                        if hp % 2 == 0:
                            st[k]["den"][hp // 2] = psp.tile(
                                [128, T], F32, name=f"den_{k}_{hp // 2}",
                                tag="den", bufs=1)
                    acc = st[k]["acc"][hp]
                    den = st[k]["den"][hp // 2]
                    vt = st[k]["vt"][s]
                    strt, stp = (s == 0), (s == NSC - 1)
                    for th in range(TH):
                        tsl = slice(512 * th, 512 * (th + 1))
                        nc.tensor.matmul(acc[0:64, tsl],
                                         vt[:, 128 * hp:128 * hp + 64],
                                         pe[:, tsl], start=strt, stop=stp,
                                         tile_position=(0, 0))
                        nc.tensor.matmul(acc[64:128, tsl],
                                         vt[:, 128 * hp + 64:128 * (hp + 1)],
                                         po[:, tsl], start=strt, stop=stp,
                                         tile_position=(0, 64))
                    pbase = 64 * (hp % 2)
                    for th in range(TH):
                        tsl = slice(512 * th, 512 * (th + 1))
                        nc.tensor.matmul(den[pbase:pbase + 1, tsl],
                                         ones1[0:sdim, :], pe[:, tsl],
                                         start=strt, stop=stp,
                                         tile_position=(0, pbase))
                        nc.tensor.matmul(den[pbase + 32:pbase + 33, tsl],
                                         ones1[0:sdim, :], po[:, tsl],
                                         start=strt, stop=stp,
                                         tile_position=(0, pbase + 32))

                def emit_evac(hp):
                    araw = app.tile([128, T], BF16, name=f"araw_{k}_{hp}",
                                    tag="araw", bufs=6)
                    nc.vector.tensor_copy(araw[:], st[k]["acc"][hp][:])
                    st[k]["araw"][hp] = araw
                    if hp % 2 == 1:
                        pr = hp // 2
                        denc = app.tile([97, T], F32, name=f"denc_{k}_{pr}",
                                        tag="denc", bufs=2,
                                        padded_shape=[128, T])
                        nc.vector.tensor_copy(denc[:], st[k]["den"][pr][0:97, :])
                        rc = app.tile([97, T], F16, name=f"rc_{k}_{pr}",
                                      tag="rc", bufs=2, padded_shape=[128, T])
                        with nc.allow_low_precision("denom recip fp16"):
                            nc.vector.reciprocal(rc[:], denc[:])
                        st[k]["rc"][pr] = rc

                def emit_normalize(hp):
                    pr = hp // 2
                    rc = st[k]["rc"][pr]
                    pbase = 64 * (hp % 2)
                    rbc = psp.tile([128, T], F32, name=f"rbc_{k}_{hp}",
                                   tag="den", bufs=1)
                    for th in range(TH):
                        tsl = slice(512 * th, 512 * (th + 1))
                        nc.tensor.matmul(
                            rbc[0:64, tsl], ones64[pbase:pbase + 1, :],
                            rc[pbase:pbase + 1, tsl], start=True, stop=True,
                            tile_position=(pbase, 0))
                        nc.tensor.matmul(
                            rbc[64:128, tsl], ones64[pbase + 32:pbase + 33, :],
                            rc[pbase + 32:pbase + 33, tsl], start=True,
                            stop=True, tile_position=(pbase + 32, 64))
                    apair = app.tile([128, T], F16, name=f"apr_{k}_{hp}",
                                     tag="apair", bufs=8)
                    nc.vector.tensor_tensor(out=apair[:],
                                            in0=st[k]["araw"][hp][:],
                                            in1=rbc[:], op=AL.mult)
                    st[k]["apair"][hp] = apair

                n_ev = len(events) + LAG
                for i in range(n_ev):
                    if i < len(events):
                        emit_scores(*events[i])
                    j = i - LAG
                    if j >= 0:
                        hp, s = events[j]
                        emit_value_den(hp, s)
                        if s == NSC - 1:
                            emit_evac(hp)
                            if hp % 2 == 1:
                                emit_normalize(hp - 1)
                                emit_normalize(hp)
                    left = n_ev - 1 - i
                    if left > 0:
                        drain(-(-len(pending) // left))  # ceil spread
                drain(len(pending))

            # ---------------- main item loop -------------------------------
            push_prep(0)
            drain(len(pending))
            for k in range(NITEMS):
                if k + 2 < NITEMS:
                    emit_input_dmas(k + 2)
                if k + 1 < NITEMS:
                    push_prep(k + 1)
                run_attention(k)
                push_proj(k)
            drain(len(pending))

    split_multi_waits(nc)
    return nc


def _prepare(inputs):
    x = np.asarray(inputs["x"], np.float32).reshape(B, C, T)
    c = np.asarray(inputs["c"], np.float32)
    gamma = np.asarray(inputs["gamma"], np.float32)
    beta = np.asarray(inputs["beta"], np.float32)
    w_qkv = np.asarray(inputs["w_qkv"], np.float32)
    b_qkv = np.asarray(inputs["b_qkv"], np.float32)
    w_c = np.asarray(inputs["w_c"], np.float32)
    b_c = np.asarray(inputs["b_c"], np.float32)
    w_p = np.asarray(inputs["w_p"], np.float32)
    b_p = np.asarray(inputs["b_p"], np.float32)

    scale = 1.0 / np.sqrt(HC)  # 0.125, exact
    wq = w_qkv[0:C] * gamma[None, :]
    wk = w_qkv[C:2 * C] * gamma[None, :] * scale
    wv = w_qkv[2 * C:3 * C] * gamma[None, :]
    qb = w_qkv[0:C] @ beta + b_qkv[0:C]
    kb = (w_qkv[C:2 * C] @ beta + b_qkv[C:2 * C]) * scale
    vb = w_qkv[2 * C:3 * C] @ beta + b_qkv[2 * C:3 * C]
    wkc = w_c[0:C] * scale
    kcb = b_c[0:C] * scale
    wvc = w_c[C:2 * C]
    vcb = b_c[C:2 * C]

    def colsplit(v):  # [512] -> [128, 4] per-channel-tile columns
        return np.ascontiguousarray(v.reshape(CT, 128).T).astype(np.float32)

    def _vbrow(vb_):
        row = np.zeros((1, HEADS * 65), np.float32)
        for h in range(HEADS):
            row[0, 65 * h:65 * h + 64] = vb_[64 * h:64 * (h + 1)]
        return row.astype(np.float16)

    G = np.zeros((128, CT, GROUPS), np.float32)
    GT = np.zeros((GROUPS, CT, 128), np.float32)
    for m in range(CT):
        for p in range(128):
            g = (m * 128 + p) // GSIZE
            G[p, m, g] = 1.0
            GT[g, m, p] = 1.0

    apply_vbias = bool(np.any(vb != 0) or np.any(vcb != 0))
    if apply_vbias and not np.allclose(vb, vcb):
        raise NotImplementedError("distinct self/cond v biases not supported")
    apply_pbias = bool(np.any(b_p != 0))

    shared = {
        "wqT": np.ascontiguousarray(wq.T).astype(np.float16),
        "wkT": np.ascontiguousarray(wk.T).astype(np.float16),
        "wvT": np.ascontiguousarray(wv.T).astype(np.float16),
        "wkcT": np.ascontiguousarray(wkc.T).astype(np.float16),
        "wvcT": np.ascontiguousarray(wvc.T).astype(np.float16),
        "wpT": np.ascontiguousarray(w_p.T).astype(np.float16),
        "G": G, "GT": GT,
        "qb": colsplit(qb), "kb": colsplit(kb), "kcb": colsplit(kcb),
        "pb": colsplit(b_p),
        "vbrow": vb[None, :].astype(np.float16),
    }
    in_maps = []
    c16 = c.astype(np.float16)
    for core in range(N_CORES):
        m = dict(shared)
        m["x_sh"] = np.ascontiguousarray(x[BPC * core:BPC * (core + 1)])
        m["c_sh"] = np.ascontiguousarray(c16[BPC * core:BPC * (core + 1)])
        in_maps.append(m)
    return in_maps, apply_vbias, apply_pbias


def run(inputs, trace=False):
    in_maps, avb, apb = _prepare(inputs)
    key = (avb, apb)
    if key not in _CACHE:
        _CACHE[key] = build_program(apply_vbias=avb, apply_pbias=apb)
    nc = _CACHE[key]
    res = run_bass_kernel_spmd(nc, in_maps, core_ids=list(range(N_CORES)),
                               trace=trace)
    out = np.concatenate([res.results[c]["out"] for c in range(N_CORES)], axis=0)
    return out.reshape(B, C, HS, WS).astype(np.float32), res


def kernel(**inputs):
    out, _ = run(inputs, trace=False)
    return out

